# revision 14
# baseline (speedup 1.0000x reference)
"""RWR kernel attention for Trainium2 (Bass/Tile), sharded over B*H on 8 cores.

Full inputs q,k,v: [2,12,1024,64] fp32. BH=24 -> 3 slices per core.
Per (b,h) slice (N=1024, D=64, scale=1/8, WINDOW=128, TOPK=32,
ALPHA=0.2, STEPS=4, LENS=0.3):
  scores = (q@k^T)/8
  y_local = softmax(where(|i-j|<=128, scores, -1e30)) @ v
  sim    = where(|i-j|<=128, -1e30, scores); top-32 per row -> P (relu'd,
           row-normalized by sum(relu(top32))+1e-9)
  R_acc  = sum_{t=0..4} alpha*(1-alpha)^t P^t;  top-32 per row of R_acc -> S
  out    = y_local + LENS * (S @ v)
Reformulated (validated to 2e-6 on CPU):
  Pc = 0.8*P built from UNSCALED sim with rp = 0.8/(rowsum_unscaled + 8e-9)
  U4 = I + Pc + Pc^2 + Pc^3 + Pc^4 via Horner W-form (W = Pc + Pc@W_prev)
  S  = (U4 >= t32(U4)) * U4   (threshold-mask == exact top-k, ties measure-0)
  G  = S + attn/0.06 ;  out = 0.06 * (G @ v)    (0.06 = LENS*ALPHA)
Top-32 via 4x(max8 + match_replace) exact full-width search on DVE.
"""

import numpy as np

import concourse.bass as bass
import concourse.bacc as bacc
import concourse.tile as tile
import concourse.mybir as mybir
from concourse.masks import make_identity
from concourse.bass_utils import run_bass_kernel_spmd

F32 = mybir.dt.float32
F32R = mybir.dt.float32r
AF = mybir.ActivationFunctionType
OP = mybir.AluOpType

NEG = -1.0e30
N = 1024
D = 64
NB = 8
BS = 128
SLICES = 3
NCORES = 8
W_MID = 384
W_EDGE = 256


def _r(ap):
    # plain fp32 matmuls: fp32r requires producers to round at writeback
    # (verifier: "not rounded to FP32r"), revisit as an optimization
    return ap


def _win(ib):
    if ib == 0:
        return 0, W_EDGE
    if ib == NB - 1:
        return N - W_EDGE, W_EDGE
    return BS * (ib - 1), W_MID


class _P:
    pass


def _build_masks(nc, P):
    pool = P.constp
    P.ident = pool.tile([BS, BS], F32, tag="ident")
    make_identity(nc, P.ident)
    P.identR = _r(P.ident)

    # B_out: 0 inside band, NEG outside (within window stripe). Local row r,
    # local col c. affine_select: expr = cm*r + coef*c + base; TRUE -> in_,
    # FALSE -> fill.
    bout_m = pool.tile([BS, W_MID], F32, tag="bout_m")
    nc.gpsimd.memset(bout_m, 0.0)
    # keep where c - r >= 0
    nc.gpsimd.affine_select(out=bout_m, in_=bout_m, pattern=[[1, W_MID]],
                            compare_op=OP.is_ge, fill=NEG, base=0,
                            channel_multiplier=-1)
    # keep where r + 256 - c >= 0
    nc.gpsimd.affine_select(out=bout_m, in_=bout_m, pattern=[[-1, W_MID]],
                            compare_op=OP.is_ge, fill=NEG, base=256,
                            channel_multiplier=1)

    bout_0 = pool.tile([BS, W_EDGE], F32, tag="bout_0")
    nc.gpsimd.memset(bout_0, 0.0)
    # keep where r + 128 - c >= 0
    nc.gpsimd.affine_select(out=bout_0, in_=bout_0, pattern=[[-1, W_EDGE]],
                            compare_op=OP.is_ge, fill=NEG, base=128,
                            channel_multiplier=1)

    bout_7 = pool.tile([BS, W_EDGE], F32, tag="bout_7")
    nc.gpsimd.memset(bout_7, 0.0)
    # keep where c - r >= 0
    nc.gpsimd.affine_select(out=bout_7, in_=bout_7, pattern=[[1, W_EDGE]],
                            compare_op=OP.is_ge, fill=NEG, base=0,
                            channel_multiplier=-1)

    # B_in: NEG inside band, 0 outside.
    bin_m = pool.tile([BS, W_MID], F32, tag="bin_m")
    nc.gpsimd.memset(bin_m, NEG)
    nc.gpsimd.affine_select(out=bin_m, in_=bin_m, pattern=[[1, W_MID]],
                            compare_op=OP.is_ge, fill=0.0, base=0,
                            channel_multiplier=-1)
    nc.gpsimd.affine_select(out=bin_m, in_=bin_m, pattern=[[-1, W_MID]],
                            compare_op=OP.is_ge, fill=0.0, base=256,
                            channel_multiplier=1)

    bin_0 = pool.tile([BS, W_EDGE], F32, tag="bin_0")
    nc.gpsimd.memset(bin_0, NEG)
    # keep where c - r + 128 >= 0
    nc.gpsimd.affine_select(out=bin_0, in_=bin_0, pattern=[[1, W_EDGE]],
                            compare_op=OP.is_ge, fill=0.0, base=128,
                            channel_multiplier=-1)
    # keep where r + 128 - c >= 0
    nc.gpsimd.affine_select(out=bin_0, in_=bin_0, pattern=[[-1, W_EDGE]],
                            compare_op=OP.is_ge, fill=0.0, base=128,
                            channel_multiplier=1)

    bin_7 = pool.tile([BS, W_EDGE], F32, tag="bin_7")
    nc.gpsimd.memset(bin_7, NEG)
    nc.gpsimd.affine_select(out=bin_7, in_=bin_7, pattern=[[1, W_EDGE]],
                            compare_op=OP.is_ge, fill=0.0, base=0,
                            channel_multiplier=-1)

    P.bout = {0: bout_0, NB - 1: bout_7}
    P.bin = {0: bin_0, NB - 1: bin_7}
    for ib in range(1, NB - 1):
        P.bout[ib] = bout_m
        P.bin[ib] = bin_m


def _search(nc, P, src, m32, width):
    """Exact top-32 of src [128,width] -> m32 [128,32] (descending within each
    8-group; m32[:,31] is the 32nd largest). src is not modified."""
    cur = src
    for rnd in range(4):
        nc.vector.max(m32[:, 8 * rnd:8 * rnd + 8], cur)
        if rnd < 3:
            nxt = P.scrp.tile([BS, width], F32, tag="s")
            nc.vector.match_replace(nxt, m32[:, 8 * rnd:8 * rnd + 8], cur, NEG)
            cur = nxt


def _emit_slice(nc, P, s):
    identR = P.identR

    # ---- DMA in ----
    q_sb = P.iop1.tile([BS, NB, D], F32, tag="q_sb")
    k_sb = P.iop1.tile([BS, NB, D], F32, tag="k_sb")
    v_sb = P.iop2.tile([BS, NB, D], F32, tag="v_sb")
    nc.sync.dma_start(q_sb, P.q_d[s].rearrange("nb p d -> p nb d"))
    nc.sync.dma_start(k_sb, P.k_d[s].rearrange("nb p d -> p nb d"))
    nc.sync.dma_start(v_sb, P.v_d[s].rearrange("nb p d -> p nb d"))

    # ---- qT/kT [64, 1024] ----
    qT = P.qkp.tile([D, N], F32, tag="qT")
    kT = P.qkp.tile([D, N], F32, tag="kT")
    for src, dst in ((q_sb, qT), (k_sb, kT)):
        for ib in range(NB):
            pst = P.pstp.tile([BS, BS], F32, tag="t")
            nc.tensor.transpose(_r(pst[0:D, :]), _r(src[:, ib, :]), identR)
            nc.scalar.activation(dst[:, ib * BS:(ib + 1) * BS], pst[0:D, :],
                                 AF.Copy)

    sumE8 = P.smallp.tile([BS, NB], F32, tag="sumE8")
    rsA8 = P.smallp.tile([BS, NB], F32, tag="rsA8")
    rp8 = P.smallp.tile([BS, NB], F32, tag="rp8")
    rEp8 = P.smallp.tile([BS, NB], F32, tag="rEp8")
    Pc = P.bigp.tile([BS, NB, N], F32, tag="Pc")
    e_tiles = []

    # ---- stage A per row-block: scores, sim, E, search, Pc ----
    for ib in range(NB):
        j0, w = _win(ib)
        psS = P.pswp.tile([BS, N], F32, tag="w")
        for h in (0, 1):
            nc.tensor.matmul(psS[:, 512 * h:512 * (h + 1)],
                             _r(qT[:, ib * BS:(ib + 1) * BS]),
                             _r(kT[:, 512 * h:512 * (h + 1)]),
                             start=True, stop=True)

        sim_t = P.simp.tile([BS, N], F32, tag="sim")
        if j0 > 0:
            nc.scalar.activation(sim_t[:, 0:j0], psS[:, 0:j0], AF.Copy)
        if j0 + w < N:
            nc.scalar.activation(sim_t[:, j0 + w:N], psS[:, j0 + w:N], AF.Copy)
        nc.vector.tensor_tensor(sim_t[:, j0:j0 + w], psS[:, j0:j0 + w],
                                P.bin[ib], OP.add)

        # local-attention numerator: exp(0.125*(scores + B_out))
        e_t = P.e8p.tile([BS, W_MID], F32, tag="E")
        nc.vector.tensor_tensor(e_t[:, 0:w], psS[:, j0:j0 + w], P.bout[ib],
                                OP.add)
        nc.scalar.activation(e_t[:, 0:w], e_t[:, 0:w], AF.Exp, scale=0.125,
                             accum_out=sumE8[:, ib:ib + 1])
        e_tiles.append(e_t)

        m32 = P.smallp.tile([BS, 32], F32, tag="m")
        _search(nc, P, sim_t, m32, N)
        dump = P.smallp.tile([BS, 32], F32, tag="dump")
        nc.scalar.activation(dump, m32, AF.Relu, accum_out=rsA8[:, ib:ib + 1])
        # rp = 0.8/(rowsum + 8e-9) = 1/(1.25*rowsum + 1e-8)
        tmp1 = P.smallp.tile([BS, 1], F32, tag="tmp1")
        nc.vector.tensor_scalar(tmp1, rsA8[:, ib:ib + 1], 1.25, 1.0e-8,
                                OP.mult, OP.add)
        nc.vector.reciprocal(rp8[:, ib:ib + 1], tmp1)
        # M = (sim >= t32) * sim ; Pc = relu(M * rp)
        M_t = P.scrp.tile([BS, N], F32, tag="s")
        nc.vector.scalar_tensor_tensor(M_t, sim_t, m32[:, 31:32], sim_t,
                                       OP.is_ge, OP.mult)
        nc.scalar.activation(Pc[:, ib, :], M_t, AF.Relu,
                             scale=rp8[:, ib:ib + 1])

    # rEp = 1/(0.06 * sumE)
    tmpE = P.smallp.tile([BS, NB], F32, tag="tmpE")
    nc.vector.tensor_scalar(tmpE, sumE8, 0.06, None, OP.mult)
    nc.vector.reciprocal(rEp8, tmpE)

    # ---- PcT ----
    PcT = P.bigp.tile([BS, NB, N], F32, tag="PcT")
    for jb in range(NB):
        for ibb in range(NB):
            pst = P.pstp.tile([BS, BS], F32, tag="t")
            nc.tensor.transpose(_r(pst), _r(Pc[:, ibb, jb * BS:(jb + 1) * BS]),
                                identR)
            nc.scalar.activation(PcT[:, jb, ibb * BS:(ibb + 1) * BS], pst,
                                 AF.Copy)

    # ---- Horner: W = Pc + Pc @ X  (U4 additionally + I on the diagonal) ----
    def horner(Xbig, wtag, with_diag):
        Wt = P.bigp.tile([BS, NB, N], F32, tag=wtag)
        for ib in range(NB):
            ps = P.pswp.tile([BS, N], F32, tag="w")
            for h in (0, 1):
                half = ps[:, 512 * h:512 * (h + 1)]
                for kb in range(NB):
                    nc.tensor.matmul(half,
                                     _r(PcT[:, kb, ib * BS:(ib + 1) * BS]),
                                     _r(Xbig[:, kb, 512 * h:512 * (h + 1)]),
                                     start=(kb == 0), stop=False)
                diag_here = with_diag and (ib // 4) == h
                nc.tensor.matmul(half, identR,
                                 _r(Pc[:, ib, 512 * h:512 * (h + 1)]),
                                 start=False, stop=not diag_here)
                if diag_here:
                    c0 = 512 * h + (ib * BS - 512 * h)
                    nc.tensor.matmul(ps[:, ib * BS:ib * BS + BS], identR,
                                     identR, start=False, stop=True,
                                     skip_group_check=True)
            nc.scalar.activation(Wt[:, ib, :], ps, AF.Copy)
        return Wt

    W2 = horner(Pc, "W2", False)
    W3 = horner(W2, "W3", False)
    U4 = horner(W3, "W2", True)

    # ---- stage B: searches, then S = (U4>=t32)*U4, G = S + attn/0.06 ----
    mB = P.smallp.tile([BS, NB * 32], F32, tag="mB")
    for rb in range(NB):
        _search(nc, P, U4[:, rb, :], mB[:, rb * 32:(rb + 1) * 32], N)
    S = P.bigp.tile([BS, NB, N], F32, tag="Pc")
    for rb in range(NB):
        j0, w = _win(rb)
        nc.vector.scalar_tensor_tensor(S[:, rb, :], U4[:, rb, :],
                                       mB[:, rb * 32 + 31:rb * 32 + 32],
                                       U4[:, rb, :], OP.is_ge, OP.mult)
        nc.vector.scalar_tensor_tensor(S[:, rb, j0:j0 + w],
                                       e_tiles[rb][:, 0:w],
                                       rEp8[:, rb:rb + 1],
                                       S[:, rb, j0:j0 + w], OP.mult, OP.add)

    # ---- ST = S^T ----
    ST = P.bigp.tile([BS, NB, N], F32, tag="W3")
    for kb in range(NB):
        for rb in range(NB):
            pst = P.pstp.tile([BS, BS], F32, tag="t")
            nc.tensor.transpose(_r(pst), _r(S[:, rb, kb * BS:(kb + 1) * BS]),
                                identR)
            nc.scalar.activation(ST[:, kb, rb * BS:(rb + 1) * BS], pst,
                                 AF.Copy)

    # ---- yT [64,1024] = V^T @ G^T, accumulated over kb ----
    yps = P.pswp.tile([BS, N], F32, tag="w")
    for h in (0, 1):
        for kb in range(NB):
            nc.tensor.matmul(yps[0:D, 512 * h:512 * (h + 1)],
                             _r(v_sb[:, kb, :]),
                             _r(ST[:, kb, 512 * h:512 * (h + 1)]),
                             start=(kb == 0), stop=(kb == NB - 1))
    yT = P.ytp.tile([D, N], F32, tag="yT")
    nc.scalar.mul(yT, yps[0:D, :], 0.06)

    # ---- transpose back + DMA out ----
    out_sb = P.iop2.tile([BS, NB, D], F32, tag="out_sb")
    for ib in range(NB):
        pst = P.pstp.tile([BS, BS], F32, tag="t")
        nc.tensor.transpose(_r(pst[:, 0:D]), _r(yT[:, ib * BS:(ib + 1) * BS]),
                            _r(P.ident[0:D, 0:D]))
        nc.scalar.activation(out_sb[:, ib, :], pst[:, 0:D], AF.Copy)
    nc.sync.dma_start(P.y_d[s].rearrange("nb p d -> p nb d"), out_sb)


_CACHE = {}


def _build_nc():
    if "nc" in _CACHE:
        return _CACHE["nc"]
    nc = bacc.Bacc(None, target_bir_lowering=False)
    P = _P()
    P.q_d = nc.declare_dram_parameter("q", [SLICES, NB, BS, D], F32, False)
    P.k_d = nc.declare_dram_parameter("k", [SLICES, NB, BS, D], F32, False)
    P.v_d = nc.declare_dram_parameter("v", [SLICES, NB, BS, D], F32, False)
    P.y_d = nc.declare_dram_parameter("y", [SLICES, NB, BS, D], F32, True)
    with tile.TileContext(nc) as tc:
        with (
            tc.tile_pool(name="constp", bufs=1) as constp,
            tc.tile_pool(name="iop1", bufs=1) as iop1,
            tc.tile_pool(name="iop2", bufs=2) as iop2,
            tc.tile_pool(name="qkp", bufs=1) as qkp,
            tc.tile_pool(name="bigp", bufs=1) as bigp,
            tc.tile_pool(name="simp", bufs=2) as simp,
            tc.tile_pool(name="scrp", bufs=2) as scrp,
            tc.tile_pool(name="e8p", bufs=8) as e8p,
            tc.tile_pool(name="ytp", bufs=2) as ytp,
            tc.tile_pool(name="smallp", bufs=2) as smallp,
            tc.tile_pool(name="pswp", bufs=2, space=bass.MemorySpace.PSUM) as pswp,
            tc.tile_pool(name="pstp", bufs=2, space=bass.MemorySpace.PSUM) as pstp,
        ):
            P.constp, P.iop1, P.iop2, P.qkp = constp, iop1, iop2, qkp
            P.bigp, P.simp, P.scrp, P.e8p = bigp, simp, scrp, e8p
            P.ytp, P.smallp, P.pswp, P.pstp = ytp, smallp, pswp, pstp
            _build_masks(nc, P)
            for s in range(SLICES):
                _emit_slice(nc, P, s)
    nc.finalize()
    _CACHE["nc"] = nc
    return nc


def _prepare(q, k, v):
    qf = np.ascontiguousarray(np.asarray(q, np.float32)).reshape(24, NB, BS, D)
    kf = np.ascontiguousarray(np.asarray(k, np.float32)).reshape(24, NB, BS, D)
    vf = np.ascontiguousarray(np.asarray(v, np.float32)).reshape(24, NB, BS, D)
    in_maps = []
    for c in range(NCORES):
        sl = slice(c * SLICES, (c + 1) * SLICES)
        in_maps.append({
            "q": np.ascontiguousarray(qf[sl]),
            "k": np.ascontiguousarray(kf[sl]),
            "v": np.ascontiguousarray(vf[sl]),
        })
    return in_maps


def _gather(results):
    outs = [np.asarray(results[c]["y"], np.float32).reshape(SLICES, N, D)
            for c in range(NCORES)]
    return np.concatenate(outs, axis=0).reshape(2, 12, N, D)


def _run_spmd(in_maps, trace=False):
    nc = _build_nc()
    return run_bass_kernel_spmd(nc, in_maps, core_ids=list(range(NCORES)),
                                trace=trace)


def kernel(q, k, v):
    in_maps = _prepare(q, k, v)
    br = _run_spmd(in_maps)
    return _gather(br.results)
